# revision 7
# baseline (speedup 1.0000x reference)
"""Trainium2 Bass kernel for DotProductAttention.

Reference computation (all fp32):
    sim_scores        = einsum('vd,bdt->bvt', q_embed, conv_feat)   # (B,V,T)
    attention_weights = softmax(sim_scores, axis=2)                 # over T
    context_vector    = einsum('bvt,bdt->bvd', weights, conv_feat)  # (B,V,D)
    returns (context_vector, attention_weights)

Shapes: q_embed (1000, 1024) f32, conv_feat (64, 1024, 800) f32.

Sharding: data-parallel over batch across 8 NeuronCores (8 batches/core,
q_embed replicated).

Per-core kernel design:
  - Q^T is built once on-chip via PE transposes and split into bf16 hi/lo
    parts (Q = Qhi + Qlo exactly, each bf16).  Scores are computed as
    Qhi@Chi + Qhi@Clo + Qlo@Chi (3 bf16 matmuls, fp32 PSUM accumulation)
    which carries ~1e-4 absolute error on the scores - effectively fp32
    for softmax purposes - at 3x the speed of native fp32 matmul.
  - Softmax over T (free dim): DVE reduce_max -> ACT exp(x-max) with
    fused row-sum (accum_out) -> DVE reciprocal + scale.  fp32 weights
    are DMA'd straight out; a bf16 copy feeds the second matmul.
  - Second einsum contracts over T, so both operands need T on the
    partition axis: W^T and C^T are produced by PE (tensor engine)
    transposes in bf16, then context = (W^T).T @ C^T in bf16.
"""

import sys

sys.path.insert(0, "/opt/trn_rl_repo")

import numpy as np

import concourse.bass as bass  # noqa: F401  (engine types pulled via nc)
import concourse.tile as tile
from concourse import bacc, mybir
from concourse.bass_utils import run_bass_kernel_spmd
from concourse.masks import make_identity

F32 = mybir.dt.float32
BF16 = mybir.dt.bfloat16

B, V, D, T = 64, 1000, 1024, 800
NCORES = 8
BPC = B // NCORES          # batches per core
VCH = 8                    # v chunks of 128 (last has 104 valid rows)
DCH = D // 128             # 8
TA = 400                   # stage-A t-chunk (PSUM bank limit 512 fp32)
NTA = T // TA              # 2
TCH = (T + 127) // 128     # 7 stage-B t-chunks (6x128 + 1x32)
EXP = mybir.ActivationFunctionType.Exp
AX = mybir.AxisListType.X


def _vrows(vc):
    return 128 if vc < VCH - 1 else V - 128 * (VCH - 1)  # 104 for last chunk


def _tlen(tc_):
    return 128 if tc_ < TCH - 1 else T - 128 * (TCH - 1)  # 32 for last chunk


def _build_kernel(nc, tc):
    q = nc.dram_tensor("q_embed", [V, D], F32, kind="ExternalInput")
    cf = nc.dram_tensor("conv_feat", [BPC, D, T], F32, kind="ExternalInput")
    out_ctx = nc.dram_tensor("context", [BPC, V, D], F32, kind="ExternalOutput")
    out_w = nc.dram_tensor("weights", [BPC, V, T], F32, kind="ExternalOutput")

    with (
        tc.tile_pool(name="const", bufs=1) as const,
        tc.tile_pool(name="qt", bufs=1) as qt_pool,
    ):
        ident32 = const.tile([128, 128], F32)
        make_identity(nc, ident32[:])
        identbf = const.tile([128, 128], BF16)
        make_identity(nc, identbf[:])

        # ---- Q^T setup: (V,D) -> bf16 hi/lo tiles [128(d), DCH, VP(v)] ----
        qhiT = qt_pool.tile([128, DCH, VCH * 128], BF16)
        qloT = qt_pool.tile([128, DCH, VCH * 128], BF16)
        # zero so padded v columns (1000..1023) give score 0 -> harmless
        nc.vector.memset(qhiT[:], 0.0)
        nc.vector.memset(qloT[:], 0.0)
        with (
            tc.tile_pool(name="qstage", bufs=2) as qstage,
            tc.tile_pool(name="qps", bufs=2, space="PSUM") as qps,
        ):
            for vc in range(VCH):
                rows = _vrows(vc)
                qsb = qstage.tile([128, D], F32)
                nc.sync.dma_start(qsb[:rows, :], q[128 * vc : 128 * vc + rows, :])
                for dc in range(DCH):
                    pq = qps.tile([128, 128], F32)
                    nc.tensor.transpose(
                        pq[:, :rows],
                        qsb[:rows, 128 * dc : 128 * (dc + 1)],
                        ident32[:rows, :rows],
                    )
                    hi = qhiT[:, dc, 128 * vc : 128 * vc + rows]
                    nc.vector.tensor_copy(hi, pq[:, :rows])
                    nc.vector.tensor_sub(
                        qloT[:, dc, 128 * vc : 128 * vc + rows], pq[:, :rows], hi
                    )

        _main_loop(nc, tc, q, cf, out_ctx, out_w, qhiT, qloT, identbf)


def _main_loop(nc, tc, q, cf, out_ctx, out_w, qhiT, qloT, identbf):
    with (
        tc.tile_pool(name="cstage", bufs=3) as cstage,
        tc.tile_pool(name="chl", bufs=2) as chl,
        tc.tile_pool(name="tposed", bufs=1) as tposed,
        tc.tile_pool(name="epool", bufs=4) as epool,
        tc.tile_pool(name="wbpool", bufs=VCH) as wbpool,
        tc.tile_pool(name="ctxpool", bufs=6) as ctxpool,
        tc.tile_pool(name="stats", bufs=3) as stats,
        tc.tile_pool(name="psA", bufs=4, space="PSUM") as psA,
        tc.tile_pool(name="psT", bufs=2, space="PSUM") as psT,
        tc.tile_pool(name="psB", bufs=2, space="PSUM") as psB,
    ):
        # ---- main per-batch loop ----
        for b in range(BPC):
            # C (fp32) is staged in D-groups of 2x128 rows and split into
            # exact bf16 hi/lo parts (C = chi + clo up to ~2^-18 rel).
            chi = chl.tile([128, DCH, T], BF16, tag="chi")
            clo = chl.tile([128, DCH, T], BF16, tag="clo")
            cfb = cf[b].rearrange("(n p) t -> p n t", p=128)
            for g in range(DCH // 2):
                cs = cstage.tile([128, 2, T], F32)
                nc.sync.dma_start(cs[:], cfb[:, 2 * g : 2 * g + 2, :])
                nc.gpsimd.tensor_copy(chi[:, 2 * g : 2 * g + 2, :], cs[:])
                nc.vector.tensor_sub(
                    clo[:, 2 * g : 2 * g + 2, :], cs[:], chi[:, 2 * g : 2 * g + 2, :]
                )

            wb_tiles = []
            # -- stage A: scores + softmax, per v-chunk --
            for vc in range(VCH):
                rows = _vrows(vc)
                vs = slice(128 * vc, 128 * (vc + 1))
                pst = []
                for ti in range(NTA):
                    ps = psA.tile([128, TA], F32)
                    pst.append(ps)
                    tsl = slice(TA * ti, TA * (ti + 1))
                    n = 0
                    for dc in range(DCH):
                        for lhs, rhs in (
                            (qhiT, chi),
                            (qhiT, clo),
                            (qloT, chi),
                        ):
                            n += 1
                            nc.tensor.matmul(
                                ps[:],
                                lhs[:, dc, vs],
                                rhs[:, dc, tsl],
                                start=(n == 1),
                                stop=(n == 3 * DCH),
                            )
                # softmax over T for this v-chunk
                m0 = stats.tile([128, 1], F32, tag="m0")
                m1 = stats.tile([128, 1], F32, tag="m1")
                nc.vector.reduce_max(out=m0[:], in_=pst[0][:], axis=AX)
                nc.vector.reduce_max(out=m1[:], in_=pst[1][:], axis=AX)
                negm = stats.tile([128, 1], F32, tag="negm")
                nc.vector.tensor_max(negm[:], m0[:], m1[:])
                nc.vector.tensor_scalar_mul(negm[:], negm[:], -1.0)
                e = epool.tile([128, T], F32)
                s0 = stats.tile([128, 1], F32, tag="s0")
                s1 = stats.tile([128, 1], F32, tag="s1")
                nc.scalar.activation(
                    out=e[:, 0:TA], in_=pst[0][:], func=EXP, bias=negm[:],
                    accum_out=s0[:],
                )
                nc.scalar.activation(
                    out=e[:, TA:T], in_=pst[1][:], func=EXP, bias=negm[:],
                    accum_out=s1[:],
                )
                rcp = stats.tile([128, 1], F32, tag="rcp")
                nc.vector.tensor_add(rcp[:], s0[:], s1[:])
                nc.vector.reciprocal(rcp[:], rcp[:])
                nc.vector.tensor_scalar_mul(e[:], e[:], rcp[:])
                nc.sync.dma_start(out_w[b, 128 * vc : 128 * vc + rows, :], e[:rows, :])
                wb = wbpool.tile([128, T], BF16)
                nc.gpsimd.tensor_copy(wb[:], e[:])
                wb_tiles.append(wb)

            # -- transposes (PE) to put T on partitions --
            # 8 [*,128] blocks land side by side in one PSUM bank, then one
            # wide DVE copy moves the whole bank to SBUF.
            ct = tposed.tile([128, TCH, D], BF16, tag="ct")
            for tch in range(TCH):
                tl = _tlen(tch)
                pt = psT.tile([128, DCH, 128], BF16, tag="pt")
                for dc in range(DCH):
                    nc.tensor.transpose(
                        pt[:tl, dc, :],
                        chi[:, dc, 128 * tch : 128 * tch + tl],
                        identbf[:],
                    )
                nc.vector.tensor_copy(
                    ct[:tl, tch, :], pt[:tl].rearrange("p a b -> p (a b)")
                )
            wt = tposed.tile([128, TCH, VCH * 128], BF16, tag="wt")
            for tch in range(TCH):
                tl = _tlen(tch)
                pt = psT.tile([128, VCH, 128], BF16, tag="pt")
                for vc in range(VCH):
                    nc.tensor.transpose(
                        pt[:tl, vc, :],
                        wb_tiles[vc][:, 128 * tch : 128 * tch + tl],
                        identbf[:],
                    )
                nc.vector.tensor_copy(
                    wt[:tl, tch, :], pt[:tl].rearrange("p a b -> p (a b)")
                )

            # -- stage B: context = W @ C^T, contraction over T --
            for vc in range(VCH):
                rows = _vrows(vc)
                vs = slice(128 * vc, 128 * (vc + 1))
                for dc2 in range(2):
                    ps = psB.tile([128, 512], F32)
                    for tch in range(TCH):
                        tl = _tlen(tch)
                        nc.tensor.matmul(
                            ps[:],
                            wt[:tl, tch, vs],
                            ct[:tl, tch, 512 * dc2 : 512 * (dc2 + 1)],
                            start=(tch == 0),
                            stop=(tch == TCH - 1),
                        )
                    octx = ctxpool.tile([128, 512], F32)
                    nc.scalar.copy(octx[:], ps[:])
                    nc.sync.dma_start(
                        out_ctx[b, 128 * vc : 128 * vc + rows,
                                512 * dc2 : 512 * (dc2 + 1)],
                        octx[:rows, :],
                    )


_NC_CACHE = None


def _get_nc():
    global _NC_CACHE
    if _NC_CACHE is None:
        nc = bacc.Bacc("TRN2", target_bir_lowering=False, debug=False,
                       num_devices=NCORES)
        with tile.TileContext(nc) as tc:
            _build_kernel(nc, tc)
        nc.compile()
        _NC_CACHE = nc
    return _NC_CACHE


def kernel(q_embed, conv_feat):
    q_embed = np.ascontiguousarray(np.asarray(q_embed), dtype=np.float32)
    conv_feat = np.ascontiguousarray(np.asarray(conv_feat), dtype=np.float32)
    assert q_embed.shape == (V, D) and conv_feat.shape == (B, D, T)

    nc = _get_nc()
    in_maps = [
        {
            "q_embed": q_embed,
            "conv_feat": np.ascontiguousarray(conv_feat[i * BPC : (i + 1) * BPC]),
        }
        for i in range(NCORES)
    ]
    try:
        res = run_bass_kernel_spmd(nc, in_maps, list(range(NCORES))).results
    except Exception:
        # transient device/RPC faults have been observed; one retry
        import time as _time

        _time.sleep(5)
        res = run_bass_kernel_spmd(nc, in_maps, list(range(NCORES))).results
    context = np.concatenate([r["context"] for r in res], axis=0)
    weights = np.concatenate([r["weights"] for r in res], axis=0)
    return context, weights


if __name__ == "__main__":
    rng = np.random.default_rng(0)
    qe = rng.standard_normal((V, D), dtype=np.float32)
    cfv = rng.standard_normal((B, D, T), dtype=np.float32)
    ctx_, w_ = kernel(qe, cfv)
    print(ctx_.shape, w_.shape, ctx_.dtype, w_.dtype)


# revision 10
# speedup vs baseline: 1.0282x; 1.0282x over previous
"""Trainium2 Bass kernel for DotProductAttention.

Reference computation (all fp32):
    sim_scores        = einsum('vd,bdt->bvt', q_embed, conv_feat)   # (B,V,T)
    attention_weights = softmax(sim_scores, axis=2)                 # over T
    context_vector    = einsum('bvt,bdt->bvd', weights, conv_feat)  # (B,V,D)
    returns (context_vector, attention_weights)

Shapes: q_embed (1000, 1024) f32, conv_feat (64, 1024, 800) f32.

Sharding: data-parallel over batch across 8 NeuronCores (8 batches/core,
q_embed replicated).

Per-core kernel design:
  - Q^T is built once on-chip via PE transposes and split into bf16 hi/lo
    parts (Q = Qhi + Qlo exactly, each bf16).  Scores are computed as
    Qhi@Chi + Qhi@Clo + Qlo@Chi (3 bf16 matmuls, fp32 PSUM accumulation)
    which carries ~1e-4 absolute error on the scores - effectively fp32
    for softmax purposes - at 3x the speed of native fp32 matmul.
  - Softmax over T (free dim): DVE reduce_max -> ACT exp(x-max) with
    fused row-sum (accum_out) -> DVE reciprocal + scale.  fp32 weights
    are DMA'd straight out; a bf16 copy feeds the second matmul.
  - Second einsum contracts over T, so both operands need T on the
    partition axis: W^T and C^T are produced by PE (tensor engine)
    transposes in bf16, then context = (W^T).T @ C^T in bf16.
"""

import sys

sys.path.insert(0, "/opt/trn_rl_repo")

import numpy as np

import concourse.bass as bass  # noqa: F401  (engine types pulled via nc)
import concourse.tile as tile
from concourse import bacc, mybir
from concourse.bass_utils import run_bass_kernel_spmd
from concourse.masks import make_identity

F32 = mybir.dt.float32
BF16 = mybir.dt.bfloat16

B, V, D, T = 64, 1000, 1024, 800
NCORES = 8
BPC = B // NCORES          # batches per core
VCH = 8                    # v chunks of 128 (last has 104 valid rows)
DCH = D // 128             # 8
TA = 400                   # stage-A t-chunk (PSUM bank limit 512 fp32)
NTA = T // TA              # 2
TCH = (T + 127) // 128     # 7 stage-B t-chunks (6x128 + 1x32)
EXP = mybir.ActivationFunctionType.Exp
AX = mybir.AxisListType.X


def _vrows(vc):
    return 128 if vc < VCH - 1 else V - 128 * (VCH - 1)  # 104 for last chunk


def _tlen(tc_):
    return 128 if tc_ < TCH - 1 else T - 128 * (TCH - 1)  # 32 for last chunk


def _build_kernel(nc, tc):
    q = nc.dram_tensor("q_embed", [V, D], F32, kind="ExternalInput")
    cf = nc.dram_tensor("conv_feat", [BPC, D, T], F32, kind="ExternalInput")
    out_ctx = nc.dram_tensor("context", [BPC, V, D], F32, kind="ExternalOutput")
    out_w = nc.dram_tensor("weights", [BPC, V, T], F32, kind="ExternalOutput")

    with (
        tc.tile_pool(name="const", bufs=1) as const,
        tc.tile_pool(name="qt", bufs=1) as qt_pool,
    ):
        ident32 = const.tile([128, 128], F32)
        make_identity(nc, ident32[:])
        identbf = const.tile([128, 128], BF16)
        make_identity(nc, identbf[:])

        # ---- Q^T setup: (V,D) -> bf16 hi/lo tiles [128(d), DCH, VP(v)] ----
        qhiT = qt_pool.tile([128, DCH, VCH * 128], BF16)
        qloT = qt_pool.tile([128, DCH, VCH * 128], BF16)
        # zero only the padded v columns (1000..1023) -> score 0, harmless
        nc.vector.memset(qhiT[:, :, V : VCH * 128], 0.0)
        nc.vector.memset(qloT[:, :, V : VCH * 128], 0.0)
        with (
            tc.tile_pool(name="qstage", bufs=2) as qstage,
            tc.tile_pool(name="qps", bufs=2, space="PSUM") as qps,
        ):
            for vc in range(VCH):
                rows = _vrows(vc)
                qsb = qstage.tile([128, D], F32)
                nc.sync.dma_start(qsb[:rows, :], q[128 * vc : 128 * vc + rows, :])
                for dc in range(DCH):
                    pq = qps.tile([128, 128], F32)
                    nc.tensor.transpose(
                        pq[:, :rows],
                        qsb[:rows, 128 * dc : 128 * (dc + 1)],
                        ident32[:rows, :rows],
                    )
                    hi = qhiT[:, dc, 128 * vc : 128 * vc + rows]
                    nc.vector.tensor_copy(hi, pq[:, :rows])
                    nc.vector.tensor_sub(
                        qloT[:, dc, 128 * vc : 128 * vc + rows], pq[:, :rows], hi
                    )

        _main_loop(nc, tc, q, cf, out_ctx, out_w, qhiT, qloT, identbf)


def _main_loop(nc, tc, q, cf, out_ctx, out_w, qhiT, qloT, identbf):
    with (
        tc.tile_pool(name="cstage", bufs=3) as cstage,
        tc.tile_pool(name="chl", bufs=2) as chl,
        tc.tile_pool(name="tposed", bufs=1) as tposed,
        tc.tile_pool(name="epool", bufs=4) as epool,
        tc.tile_pool(name="wbpool", bufs=VCH) as wbpool,
        tc.tile_pool(name="ctxpool", bufs=6) as ctxpool,
        tc.tile_pool(name="stats", bufs=3) as stats,
        tc.tile_pool(name="rcps", bufs=VCH + 1) as rcps,
        tc.tile_pool(name="psA", bufs=4, space="PSUM") as psA,
        tc.tile_pool(name="psT", bufs=2, space="PSUM") as psT,
        tc.tile_pool(name="psB", bufs=2, space="PSUM") as psB,
    ):
        # ---- main per-batch loop ----
        for b in range(BPC):
            # C (fp32) is staged in D-groups of 2x128 rows and split into
            # exact bf16 hi/lo parts (C = chi + clo up to ~2^-18 rel).
            chi = chl.tile([128, DCH, T], BF16, tag="chi")
            clo = chl.tile([128, DCH, T], BF16, tag="clo")
            cfb = cf[b].rearrange("(n p) t -> p n t", p=128)
            for g in range(DCH // 2):
                cs = cstage.tile([128, 2, T], F32)
                nc.sync.dma_start(cs[:], cfb[:, 2 * g : 2 * g + 2, :])
                nc.gpsimd.tensor_copy(chi[:, 2 * g : 2 * g + 2, :], cs[:])
                nc.vector.tensor_sub(
                    clo[:, 2 * g : 2 * g + 2, :], cs[:], chi[:, 2 * g : 2 * g + 2, :]
                )

            wb_tiles = []
            rcp_tiles = []
            # -- stage A: scores + softmax, per v-chunk --
            for vc in range(VCH):
                rows = _vrows(vc)
                vs = slice(128 * vc, 128 * (vc + 1))
                pst = []
                for ti in range(NTA):
                    ps = psA.tile([128, TA], F32)
                    pst.append(ps)
                    tsl = slice(TA * ti, TA * (ti + 1))
                    n = 0
                    for dc in range(DCH):
                        for lhs, rhs in (
                            (qhiT, chi),
                            (qhiT, clo),
                            (qloT, chi),
                        ):
                            n += 1
                            nc.tensor.matmul(
                                ps[:],
                                lhs[:, dc, vs],
                                rhs[:, dc, tsl],
                                start=(n == 1),
                                stop=(n == 3 * DCH),
                            )
                # softmax over T for this v-chunk
                m0 = stats.tile([128, 1], F32, tag="m0")
                m1 = stats.tile([128, 1], F32, tag="m1")
                nc.vector.reduce_max(out=m0[:], in_=pst[0][:], axis=AX)
                nc.vector.reduce_max(out=m1[:], in_=pst[1][:], axis=AX)
                negm = stats.tile([128, 1], F32, tag="negm")
                nc.vector.tensor_max(negm[:], m0[:], m1[:])
                nc.vector.tensor_scalar_mul(negm[:], negm[:], -1.0)
                e = epool.tile([128, T], F32)
                s0 = stats.tile([128, 1], F32, tag="s0")
                s1 = stats.tile([128, 1], F32, tag="s1")
                nc.scalar.activation(
                    out=e[:, 0:TA], in_=pst[0][:], func=EXP, bias=negm[:],
                    accum_out=s0[:],
                )
                nc.scalar.activation(
                    out=e[:, TA:T], in_=pst[1][:], func=EXP, bias=negm[:],
                    accum_out=s1[:],
                )
                # stage B uses UNNORMALIZED exp in bf16 (available right after
                # exp); 1/sum is folded into the context PSUM->SBUF copy as a
                # per-partition scale, so the sum/reciprocal/normalize chain
                # stays off the PE critical path.
                wb = wbpool.tile([128, T], BF16)
                nc.gpsimd.tensor_copy(wb[:], e[:])
                wb_tiles.append(wb)
                rcp = rcps.tile([128, 1], F32, tag="rcp")
                nc.vector.tensor_add(rcp[:], s0[:], s1[:])
                nc.vector.reciprocal(rcp[:], rcp[:])
                rcp_tiles.append(rcp)
                nc.vector.tensor_scalar_mul(e[:], e[:], rcp[:])
                nc.sync.dma_start(out_w[b, 128 * vc : 128 * vc + rows, :], e[:rows, :])

            # -- transposes (PE) to put T on partitions --
            # 8 [*,128] blocks land side by side in one PSUM bank, then one
            # wide DVE copy moves the whole bank to SBUF.
            ct = tposed.tile([128, TCH, D], BF16, tag="ct")
            for tch in range(TCH):
                tl = _tlen(tch)
                pt = psT.tile([128, DCH, 128], BF16, tag="pt")
                for dc in range(DCH):
                    nc.tensor.transpose(
                        pt[:tl, dc, :],
                        chi[:, dc, 128 * tch : 128 * tch + tl],
                        identbf[:],
                    )
                nc.vector.tensor_copy(
                    ct[:tl, tch, :], pt[:tl].rearrange("p a b -> p (a b)")
                )
            wt = tposed.tile([128, TCH, VCH * 128], BF16, tag="wt")
            for tch in range(TCH):
                tl = _tlen(tch)
                pt = psT.tile([128, VCH, 128], BF16, tag="pt")
                for vc in range(VCH):
                    nc.tensor.transpose(
                        pt[:tl, vc, :],
                        wb_tiles[vc][:, 128 * tch : 128 * tch + tl],
                        identbf[:],
                    )
                nc.vector.tensor_copy(
                    wt[:tl, tch, :], pt[:tl].rearrange("p a b -> p (a b)")
                )

            # -- stage B: context = W @ C^T, contraction over T --
            for vc in range(VCH):
                rows = _vrows(vc)
                vs = slice(128 * vc, 128 * (vc + 1))
                for dc2 in range(2):
                    ps = psB.tile([128, 512], F32)
                    for tch in range(TCH):
                        tl = _tlen(tch)
                        nc.tensor.matmul(
                            ps[:],
                            wt[:tl, tch, vs],
                            ct[:tl, tch, 512 * dc2 : 512 * (dc2 + 1)],
                            start=(tch == 0),
                            stop=(tch == TCH - 1),
                        )
                    octx = ctxpool.tile([128, 512], F32)
                    nc.scalar.mul(octx[:], ps[:], rcp_tiles[vc][:])
                    nc.sync.dma_start(
                        out_ctx[b, 128 * vc : 128 * vc + rows,
                                512 * dc2 : 512 * (dc2 + 1)],
                        octx[:rows, :],
                    )


_NC_CACHE = None


def _get_nc():
    global _NC_CACHE
    if _NC_CACHE is None:
        nc = bacc.Bacc("TRN2", target_bir_lowering=False, debug=False,
                       num_devices=NCORES)
        with tile.TileContext(nc) as tc:
            _build_kernel(nc, tc)
        nc.compile()
        _NC_CACHE = nc
    return _NC_CACHE


def kernel(q_embed, conv_feat):
    q_embed = np.ascontiguousarray(np.asarray(q_embed), dtype=np.float32)
    conv_feat = np.ascontiguousarray(np.asarray(conv_feat), dtype=np.float32)
    assert q_embed.shape == (V, D) and conv_feat.shape == (B, D, T)

    nc = _get_nc()
    in_maps = [
        {
            "q_embed": q_embed,
            "conv_feat": np.ascontiguousarray(conv_feat[i * BPC : (i + 1) * BPC]),
        }
        for i in range(NCORES)
    ]
    try:
        res = run_bass_kernel_spmd(nc, in_maps, list(range(NCORES))).results
    except Exception:
        # transient device/RPC faults have been observed; one retry
        import time as _time

        _time.sleep(5)
        res = run_bass_kernel_spmd(nc, in_maps, list(range(NCORES))).results
    context = np.concatenate([r["context"] for r in res], axis=0)
    weights = np.concatenate([r["weights"] for r in res], axis=0)
    return context, weights


if __name__ == "__main__":
    rng = np.random.default_rng(0)
    qe = rng.standard_normal((V, D), dtype=np.float32)
    cfv = rng.standard_normal((B, D, T), dtype=np.float32)
    ctx_, w_ = kernel(qe, cfv)
    print(ctx_.shape, w_.shape, ctx_.dtype, w_.dtype)


# revision 14
# speedup vs baseline: 1.0575x; 1.0286x over previous
"""Trainium2 Bass kernel for DotProductAttention.

Reference computation (all fp32):
    sim_scores        = einsum('vd,bdt->bvt', q_embed, conv_feat)   # (B,V,T)
    attention_weights = softmax(sim_scores, axis=2)                 # over T
    context_vector    = einsum('bvt,bdt->bvd', weights, conv_feat)  # (B,V,D)
    returns (context_vector, attention_weights)

Shapes: q_embed (1000, 1024) f32, conv_feat (64, 1024, 800) f32.

Sharding: data-parallel over batch across 8 NeuronCores (8 batches/core,
q_embed replicated).

Per-core kernel design:
  - Q^T is built once on-chip via PE transposes and split into bf16 hi/lo
    parts (Q = Qhi + Qlo exactly, each bf16).  Scores are computed as
    Qhi@Chi + Qhi@Clo + Qlo@Chi (3 bf16 matmuls, fp32 PSUM accumulation)
    which carries ~1e-4 absolute error on the scores - effectively fp32
    for softmax purposes - at 3x the speed of native fp32 matmul.
  - Softmax over T (free dim): DVE reduce_max -> ACT exp(x-max) with
    fused row-sum (accum_out) -> DVE reciprocal + scale.  fp32 weights
    are DMA'd straight out; a bf16 copy feeds the second matmul.
  - Second einsum contracts over T, so both operands need T on the
    partition axis: W^T and C^T are produced by PE (tensor engine)
    transposes in bf16, then context = (W^T).T @ C^T in bf16.
"""

import sys

sys.path.insert(0, "/opt/trn_rl_repo")

import numpy as np

import concourse.bass as bass  # noqa: F401  (engine types pulled via nc)
import concourse.tile as tile
from concourse import bacc, mybir
from concourse.bass_utils import run_bass_kernel_spmd
from concourse.masks import make_identity

F32 = mybir.dt.float32
BF16 = mybir.dt.bfloat16

B, V, D, T = 64, 1000, 1024, 800
NCORES = 8
BPC = B // NCORES          # batches per core
VCH = 8                    # v chunks of 128 (last has 104 valid rows)
DCH = D // 128             # 8
TA = 400                   # stage-A t-chunk (PSUM bank limit 512 fp32)
NTA = T // TA              # 2
TCH = (T + 127) // 128     # 7 stage-B t-chunks (6x128 + 1x32)
EXP = mybir.ActivationFunctionType.Exp
AX = mybir.AxisListType.X


def _vrows(vc):
    return 128 if vc < VCH - 1 else V - 128 * (VCH - 1)  # 104 for last chunk


def _tlen(tc_):
    return 128 if tc_ < TCH - 1 else T - 128 * (TCH - 1)  # 32 for last chunk


def _build_kernel(nc, tc):
    q = nc.dram_tensor("q_embed", [V, D], F32, kind="ExternalInput")
    cf = nc.dram_tensor("conv_feat", [BPC, D, T], F32, kind="ExternalInput")
    out_ctx = nc.dram_tensor("context", [BPC, V, D], F32, kind="ExternalOutput")
    out_w = nc.dram_tensor("weights", [BPC, V, T], F32, kind="ExternalOutput")

    with (
        tc.tile_pool(name="const", bufs=1) as const,
        tc.tile_pool(name="qt", bufs=1) as qt_pool,
    ):
        ident32 = const.tile([128, 128], F32)
        make_identity(nc, ident32[:])
        identbf = const.tile([128, 128], BF16)
        make_identity(nc, identbf[:])

        # ---- Q^T hi/lo tiles [128(d), DCH, VP(v)]; filled lazily inside the
        # main loop (interleaved with batch-0 stage A so PE never waits for
        # the full q DMA before starting real work) ----
        qhiT = qt_pool.tile([128, DCH, VCH * 128], BF16)
        qloT = qt_pool.tile([128, DCH, VCH * 128], BF16)
        # zero only the padded v columns (1000..1023) -> score 0, harmless
        nc.vector.memset(qhiT[:, :, V : VCH * 128], 0.0)
        nc.vector.memset(qloT[:, :, V : VCH * 128], 0.0)

        _main_loop(nc, tc, q, cf, out_ctx, out_w, qhiT, qloT, identbf, ident32)


def _main_loop(nc, tc, q, cf, out_ctx, out_w, qhiT, qloT, identbf, ident32):
    with (
        tc.tile_pool(name="cstage", bufs=3) as cstage,
        tc.tile_pool(name="chl", bufs=2) as chl,
        tc.tile_pool(name="tposed", bufs=1) as tposed,
        tc.tile_pool(name="epool", bufs=4) as epool,
        tc.tile_pool(name="wbpool", bufs=VCH) as wbpool,
        tc.tile_pool(name="ctxpool", bufs=6) as ctxpool,
        tc.tile_pool(name="stats", bufs=3) as stats,
        tc.tile_pool(name="rcps", bufs=VCH + 1) as rcps,
        tc.tile_pool(name="psA", bufs=4, space="PSUM") as psA,
        tc.tile_pool(name="psT", bufs=2, space="PSUM") as psT,
        tc.tile_pool(name="psB", bufs=2, space="PSUM") as psB,
    ):
        # ---- main per-batch loop ----
        for b in range(BPC):
            # C (fp32) is staged in D-groups of 2x128 rows and split into
            # exact bf16 hi/lo parts (C = chi + clo up to ~2^-18 rel).
            chi = chl.tile([128, DCH, T], BF16, tag="chi")
            clo = chl.tile([128, DCH, T], BF16, tag="clo")
            cfb = cf[b].rearrange("(n p) t -> p n t", p=128)
            for g in range(DCH // 2):
                cs = cstage.tile([128, 2, T], F32)
                nc.sync.dma_start(cs[:], cfb[:, 2 * g : 2 * g + 2, :])
                nc.gpsimd.tensor_copy(chi[:, 2 * g : 2 * g + 2, :], cs[:])
                nc.vector.tensor_sub(
                    clo[:, 2 * g : 2 * g + 2, :], cs[:], chi[:, 2 * g : 2 * g + 2, :]
                )

            wb_tiles = []
            rcp_tiles = []
            # -- stage A: scores + softmax, per v-chunk --
            for vc in range(VCH):
                rows = _vrows(vc)
                vs = slice(128 * vc, 128 * (vc + 1))
                if b == 0:
                    # Q^T setup for this v-chunk: transpose q rows into bf16
                    # hi/lo parts right before they are first needed, so the
                    # q DMA streams in parallel with batch-0 compute.
                    qsb = cstage.tile([128, D], F32, tag="qsb")
                    nc.sync.dma_start(
                        qsb[:rows, :], q[128 * vc : 128 * vc + rows, :]
                    )
                    for dc in range(DCH):
                        pq = psT.tile([128, 128], F32, tag="pt")
                        nc.tensor.transpose(
                            pq[:, :rows],
                            qsb[:rows, 128 * dc : 128 * (dc + 1)],
                            ident32[:rows, :rows],
                        )
                        hi = qhiT[:, dc, 128 * vc : 128 * vc + rows]
                        nc.vector.tensor_copy(hi, pq[:, :rows])
                        nc.vector.tensor_sub(
                            qloT[:, dc, 128 * vc : 128 * vc + rows],
                            pq[:, :rows], hi,
                        )
                pst = []
                for ti in range(NTA):
                    ps = psA.tile([128, TA], F32)
                    pst.append(ps)
                    tsl = slice(TA * ti, TA * (ti + 1))
                    n = 0
                    for dc in range(DCH):
                        for lhs, rhs in (
                            (qhiT, chi),
                            (qhiT, clo),
                            (qloT, chi),
                        ):
                            n += 1
                            nc.tensor.matmul(
                                ps[:],
                                lhs[:, dc, vs],
                                rhs[:, dc, tsl],
                                start=(n == 1),
                                stop=(n == 3 * DCH),
                            )
                # softmax over T for this v-chunk
                m0 = stats.tile([128, 1], F32, tag="m0")
                m1 = stats.tile([128, 1], F32, tag="m1")
                nc.vector.reduce_max(out=m0[:], in_=pst[0][:], axis=AX)
                nc.vector.reduce_max(out=m1[:], in_=pst[1][:], axis=AX)
                negm = stats.tile([128, 1], F32, tag="negm")
                nc.vector.tensor_max(negm[:], m0[:], m1[:])
                nc.vector.tensor_scalar_mul(negm[:], negm[:], -1.0)
                e = epool.tile([128, T], F32)
                s0 = stats.tile([128, 1], F32, tag="s0")
                s1 = stats.tile([128, 1], F32, tag="s1")
                nc.scalar.activation(
                    out=e[:, 0:TA], in_=pst[0][:], func=EXP, bias=negm[:],
                    accum_out=s0[:],
                )
                nc.scalar.activation(
                    out=e[:, TA:T], in_=pst[1][:], func=EXP, bias=negm[:],
                    accum_out=s1[:],
                )
                # stage B uses UNNORMALIZED exp in bf16 (available right after
                # exp); 1/sum is folded into the context PSUM->SBUF copy as a
                # per-partition scale, so the sum/reciprocal/normalize chain
                # stays off the PE critical path.
                wb = wbpool.tile([128, T], BF16)
                nc.gpsimd.tensor_copy(wb[:], e[:])
                wb_tiles.append(wb)
                rcp = rcps.tile([128, 1], F32, tag="rcp")
                nc.vector.tensor_add(rcp[:], s0[:], s1[:])
                nc.vector.reciprocal(rcp[:], rcp[:])
                rcp_tiles.append(rcp)
                nc.vector.tensor_scalar_mul(e[:], e[:], rcp[:])
                nc.sync.dma_start(out_w[b, 128 * vc : 128 * vc + rows, :], e[:rows, :])

            # -- transposes (PE) to put T on partitions --
            # 8 [*,128] blocks land side by side in one PSUM bank, then one
            # wide DVE copy moves the whole bank to SBUF.
            ct = tposed.tile([128, TCH, D], BF16, tag="ct")
            for tch in range(TCH):
                tl = _tlen(tch)
                pt = psT.tile([128, DCH, 128], BF16, tag="pt")
                for dc in range(DCH):
                    nc.tensor.transpose(
                        pt[:tl, dc, :],
                        chi[:, dc, 128 * tch : 128 * tch + tl],
                        identbf[:],
                    )
                nc.vector.tensor_copy(
                    ct[:tl, tch, :], pt[:tl].rearrange("p a b -> p (a b)")
                )
            wt = tposed.tile([128, TCH, VCH * 128], BF16, tag="wt")
            for tch in range(TCH):
                tl = _tlen(tch)
                pt = psT.tile([128, VCH, 128], BF16, tag="pt")
                for vc in range(VCH):
                    nc.tensor.transpose(
                        pt[:tl, vc, :],
                        wb_tiles[vc][:, 128 * tch : 128 * tch + tl],
                        identbf[:],
                    )
                nc.vector.tensor_copy(
                    wt[:tl, tch, :], pt[:tl].rearrange("p a b -> p (a b)")
                )

            # -- stage B: context = W @ C^T, contraction over T --
            for vc in range(VCH):
                rows = _vrows(vc)
                vs = slice(128 * vc, 128 * (vc + 1))
                for dc2 in range(2):
                    ps = psB.tile([128, 512], F32)
                    for tch in range(TCH):
                        tl = _tlen(tch)
                        nc.tensor.matmul(
                            ps[:],
                            wt[:tl, tch, vs],
                            ct[:tl, tch, 512 * dc2 : 512 * (dc2 + 1)],
                            start=(tch == 0),
                            stop=(tch == TCH - 1),
                        )
                    octx = ctxpool.tile([128, 512], F32)
                    nc.scalar.mul(octx[:], ps[:], rcp_tiles[vc][:])
                    nc.sync.dma_start(
                        out_ctx[b, 128 * vc : 128 * vc + rows,
                                512 * dc2 : 512 * (dc2 + 1)],
                        octx[:rows, :],
                    )


_NC_CACHE = None


def _get_nc():
    global _NC_CACHE
    if _NC_CACHE is None:
        nc = bacc.Bacc("TRN2", target_bir_lowering=False, debug=False,
                       num_devices=NCORES)
        with tile.TileContext(nc) as tc:
            _build_kernel(nc, tc)
        nc.compile()
        _NC_CACHE = nc
    return _NC_CACHE


def kernel(q_embed, conv_feat):
    q_embed = np.ascontiguousarray(np.asarray(q_embed), dtype=np.float32)
    conv_feat = np.ascontiguousarray(np.asarray(conv_feat), dtype=np.float32)
    assert q_embed.shape == (V, D) and conv_feat.shape == (B, D, T)

    nc = _get_nc()
    in_maps = [
        {
            "q_embed": q_embed,
            "conv_feat": np.ascontiguousarray(conv_feat[i * BPC : (i + 1) * BPC]),
        }
        for i in range(NCORES)
    ]
    try:
        res = run_bass_kernel_spmd(nc, in_maps, list(range(NCORES))).results
    except Exception:
        # transient device/RPC faults have been observed; one retry
        import time as _time

        _time.sleep(5)
        res = run_bass_kernel_spmd(nc, in_maps, list(range(NCORES))).results
    context = np.concatenate([r["context"] for r in res], axis=0)
    weights = np.concatenate([r["weights"] for r in res], axis=0)
    return context, weights


if __name__ == "__main__":
    rng = np.random.default_rng(0)
    qe = rng.standard_normal((V, D), dtype=np.float32)
    cfv = rng.standard_normal((B, D, T), dtype=np.float32)
    ctx_, w_ = kernel(qe, cfv)
    print(ctx_.shape, w_.shape, ctx_.dtype, w_.dtype)


# revision 16
# speedup vs baseline: 1.0649x; 1.0070x over previous
"""Trainium2 Bass kernel for DotProductAttention.

Reference computation (all fp32):
    sim_scores        = einsum('vd,bdt->bvt', q_embed, conv_feat)   # (B,V,T)
    attention_weights = softmax(sim_scores, axis=2)                 # over T
    context_vector    = einsum('bvt,bdt->bvd', weights, conv_feat)  # (B,V,D)
    returns (context_vector, attention_weights)

Shapes: q_embed (1000, 1024) f32, conv_feat (64, 1024, 800) f32.

Sharding: data-parallel over batch across 8 NeuronCores (8 batches/core,
q_embed replicated).

Per-core kernel design:
  - Q^T is built once on-chip via PE transposes and split into bf16 hi/lo
    parts (Q = Qhi + Qlo exactly, each bf16).  Scores are computed as
    Qhi@Chi + Qhi@Clo + Qlo@Chi (3 bf16 matmuls, fp32 PSUM accumulation)
    which carries ~1e-4 absolute error on the scores - effectively fp32
    for softmax purposes - at 3x the speed of native fp32 matmul.
  - Softmax over T (free dim): DVE reduce_max -> ACT exp(x-max) with
    fused row-sum (accum_out) -> DVE reciprocal + scale.  fp32 weights
    are DMA'd straight out; a bf16 copy feeds the second matmul.
  - Second einsum contracts over T, so both operands need T on the
    partition axis: W^T and C^T are produced by PE (tensor engine)
    transposes in bf16, then context = (W^T).T @ C^T in bf16.
"""

import sys

sys.path.insert(0, "/opt/trn_rl_repo")

import numpy as np

import concourse.bass as bass  # noqa: F401  (engine types pulled via nc)
import concourse.tile as tile
from concourse import bacc, mybir
from concourse.bass_utils import run_bass_kernel_spmd
from concourse.masks import make_identity

F32 = mybir.dt.float32
BF16 = mybir.dt.bfloat16

B, V, D, T = 64, 1000, 1024, 800
NCORES = 8
BPC = B // NCORES          # batches per core
VCH = 8                    # v chunks of 128 (last has 104 valid rows)
DCH = D // 128             # 8
TA = 400                   # stage-A t-chunk (PSUM bank limit 512 fp32)
NTA = T // TA              # 2
TCH = (T + 127) // 128     # 7 stage-B t-chunks (6x128 + 1x32)
EXP = mybir.ActivationFunctionType.Exp
AX = mybir.AxisListType.X


def _vrows(vc):
    return 128 if vc < VCH - 1 else V - 128 * (VCH - 1)  # 104 for last chunk


def _tlen(tc_):
    return 128 if tc_ < TCH - 1 else T - 128 * (TCH - 1)  # 32 for last chunk


def _build_kernel(nc, tc):
    q = nc.dram_tensor("q_embed", [V, D], F32, kind="ExternalInput")
    cf = nc.dram_tensor("conv_feat", [BPC, D, T], F32, kind="ExternalInput")
    out_ctx = nc.dram_tensor("context", [BPC, V, D], F32, kind="ExternalOutput")
    out_w = nc.dram_tensor("weights", [BPC, V, T], F32, kind="ExternalOutput")

    with (
        tc.tile_pool(name="const", bufs=1) as const,
        tc.tile_pool(name="qt", bufs=1) as qt_pool,
    ):
        ident32 = const.tile([128, 128], F32)
        make_identity(nc, ident32[:])
        identbf = const.tile([128, 128], BF16)
        make_identity(nc, identbf[:])

        # ---- Q^T hi/lo tiles [128(d), DCH, VP(v)]; filled lazily inside the
        # main loop (interleaved with batch-0 stage A so PE never waits for
        # the full q DMA before starting real work) ----
        qhiT = qt_pool.tile([128, DCH, VCH * 128], BF16)
        qloT = qt_pool.tile([128, DCH, VCH * 128], BF16)
        # zero only the padded v columns (1000..1023) -> score 0, harmless
        nc.vector.memset(qhiT[:, :, V : VCH * 128], 0.0)
        nc.vector.memset(qloT[:, :, V : VCH * 128], 0.0)

        _main_loop(nc, tc, q, cf, out_ctx, out_w, qhiT, qloT, identbf, ident32)


def _main_loop(nc, tc, q, cf, out_ctx, out_w, qhiT, qloT, identbf, ident32):
    with (
        tc.tile_pool(name="cstage", bufs=3) as cstage,
        tc.tile_pool(name="chl", bufs=2) as chl,
        tc.tile_pool(name="tposed", bufs=1) as tposed,
        tc.tile_pool(name="epool", bufs=4) as epool,
        tc.tile_pool(name="wbpool", bufs=VCH) as wbpool,
        tc.tile_pool(name="ctxpool", bufs=6) as ctxpool,
        tc.tile_pool(name="stats", bufs=3) as stats,
        tc.tile_pool(name="rcps", bufs=VCH + 1) as rcps,
        tc.tile_pool(name="psA", bufs=4, space="PSUM") as psA,
        tc.tile_pool(name="psT", bufs=2, space="PSUM") as psT,
        tc.tile_pool(name="psB", bufs=2, space="PSUM") as psB,
    ):
        # ---- main per-batch loop ----
        for b in range(BPC):
            # C (fp32) is staged in D-groups of 2x128 rows and split into
            # exact bf16 hi/lo parts (C = chi + clo up to ~2^-18 rel).
            chi = chl.tile([128, DCH, T], BF16, tag="chi")
            clo = chl.tile([128, DCH, T], BF16, tag="clo")
            cfb = cf[b].rearrange("(n p) t -> p n t", p=128)
            qsb0 = None
            if b == 0:
                # issue the first q chunk ahead of the big C staging DMAs so
                # batch-0's first transposes are not queued behind ~9us of C
                qsb0 = cstage.tile([128, D], F32, tag="qsb")
                nc.sync.dma_start(qsb0[:_vrows(0), :], q[0 : _vrows(0), :])
            for g in range(DCH // 2):
                cs = cstage.tile([128, 2, T], F32)
                nc.sync.dma_start(cs[:], cfb[:, 2 * g : 2 * g + 2, :])
                nc.gpsimd.tensor_copy(chi[:, 2 * g : 2 * g + 2, :], cs[:])
                nc.vector.tensor_sub(
                    clo[:, 2 * g : 2 * g + 2, :], cs[:], chi[:, 2 * g : 2 * g + 2, :]
                )

            wb_tiles = []
            rcp_tiles = []
            # -- stage A: scores + softmax, per v-chunk --
            for vc in range(VCH):
                rows = _vrows(vc)
                vs = slice(128 * vc, 128 * (vc + 1))
                if b == 0:
                    # Q^T setup for this v-chunk: transpose q rows into bf16
                    # hi/lo parts right before they are first needed, so the
                    # q DMA streams in parallel with batch-0 compute.
                    if vc == 0:
                        qsb = qsb0
                    else:
                        qsb = cstage.tile([128, D], F32, tag="qsb")
                        nc.sync.dma_start(
                            qsb[:rows, :], q[128 * vc : 128 * vc + rows, :]
                        )
                    for dc in range(DCH):
                        pq = psT.tile([128, 128], F32, tag="pt")
                        nc.tensor.transpose(
                            pq[:, :rows],
                            qsb[:rows, 128 * dc : 128 * (dc + 1)],
                            ident32[:rows, :rows],
                        )
                        hi = qhiT[:, dc, 128 * vc : 128 * vc + rows]
                        nc.vector.tensor_copy(hi, pq[:, :rows])
                        nc.vector.tensor_sub(
                            qloT[:, dc, 128 * vc : 128 * vc + rows],
                            pq[:, :rows], hi,
                        )
                pst = []
                for ti in range(NTA):
                    ps = psA.tile([128, TA], F32)
                    pst.append(ps)
                    tsl = slice(TA * ti, TA * (ti + 1))
                    n = 0
                    for dc in range(DCH):
                        for lhs, rhs in (
                            (qhiT, chi),
                            (qhiT, clo),
                            (qloT, chi),
                        ):
                            n += 1
                            nc.tensor.matmul(
                                ps[:],
                                lhs[:, dc, vs],
                                rhs[:, dc, tsl],
                                start=(n == 1),
                                stop=(n == 3 * DCH),
                            )
                # softmax over T for this v-chunk
                m0 = stats.tile([128, 1], F32, tag="m0")
                m1 = stats.tile([128, 1], F32, tag="m1")
                nc.vector.reduce_max(out=m0[:], in_=pst[0][:], axis=AX)
                nc.vector.reduce_max(out=m1[:], in_=pst[1][:], axis=AX)
                negm = stats.tile([128, 1], F32, tag="negm")
                nc.vector.tensor_max(negm[:], m0[:], m1[:])
                nc.vector.tensor_scalar_mul(negm[:], negm[:], -1.0)
                e = epool.tile([128, T], F32)
                s0 = stats.tile([128, 1], F32, tag="s0")
                s1 = stats.tile([128, 1], F32, tag="s1")
                nc.scalar.activation(
                    out=e[:, 0:TA], in_=pst[0][:], func=EXP, bias=negm[:],
                    accum_out=s0[:],
                )
                nc.scalar.activation(
                    out=e[:, TA:T], in_=pst[1][:], func=EXP, bias=negm[:],
                    accum_out=s1[:],
                )
                # stage B uses UNNORMALIZED exp in bf16 (available right after
                # exp); 1/sum is folded into the context PSUM->SBUF copy as a
                # per-partition scale, so the sum/reciprocal/normalize chain
                # stays off the PE critical path.
                wb = wbpool.tile([128, T], BF16)
                nc.gpsimd.tensor_copy(wb[:], e[:])
                wb_tiles.append(wb)
                rcp = rcps.tile([128, 1], F32, tag="rcp")
                nc.vector.tensor_add(rcp[:], s0[:], s1[:])
                nc.vector.reciprocal(rcp[:], rcp[:])
                rcp_tiles.append(rcp)
                nc.vector.tensor_scalar_mul(e[:], e[:], rcp[:])
                nc.sync.dma_start(out_w[b, 128 * vc : 128 * vc + rows, :], e[:rows, :])

            # -- transposes (PE) to put T on partitions --
            # 8 [*,128] blocks land side by side in one PSUM bank, then one
            # wide DVE copy moves the whole bank to SBUF.
            ct = tposed.tile([128, TCH, D], BF16, tag="ct")
            for tch in range(TCH):
                tl = _tlen(tch)
                pt = psT.tile([128, DCH, 128], BF16, tag="pt")
                for dc in range(DCH):
                    nc.tensor.transpose(
                        pt[:tl, dc, :],
                        chi[:, dc, 128 * tch : 128 * tch + tl],
                        identbf[:],
                    )
                nc.vector.tensor_copy(
                    ct[:tl, tch, :], pt[:tl].rearrange("p a b -> p (a b)")
                )
            wt = tposed.tile([128, TCH, VCH * 128], BF16, tag="wt")
            for tch in range(TCH):
                tl = _tlen(tch)
                pt = psT.tile([128, VCH, 128], BF16, tag="pt")
                for vc in range(VCH):
                    nc.tensor.transpose(
                        pt[:tl, vc, :],
                        wb_tiles[vc][:, 128 * tch : 128 * tch + tl],
                        identbf[:],
                    )
                nc.vector.tensor_copy(
                    wt[:tl, tch, :], pt[:tl].rearrange("p a b -> p (a b)")
                )

            # -- stage B: context = W @ C^T, contraction over T --
            for vc in range(VCH):
                rows = _vrows(vc)
                vs = slice(128 * vc, 128 * (vc + 1))
                for dc2 in range(2):
                    ps = psB.tile([128, 512], F32)
                    for tch in range(TCH):
                        tl = _tlen(tch)
                        nc.tensor.matmul(
                            ps[:],
                            wt[:tl, tch, vs],
                            ct[:tl, tch, 512 * dc2 : 512 * (dc2 + 1)],
                            start=(tch == 0),
                            stop=(tch == TCH - 1),
                        )
                    octx = ctxpool.tile([128, 512], F32)
                    nc.scalar.mul(octx[:], ps[:], rcp_tiles[vc][:])
                    nc.sync.dma_start(
                        out_ctx[b, 128 * vc : 128 * vc + rows,
                                512 * dc2 : 512 * (dc2 + 1)],
                        octx[:rows, :],
                    )


_NC_CACHE = None


def _get_nc():
    global _NC_CACHE
    if _NC_CACHE is None:
        nc = bacc.Bacc("TRN2", target_bir_lowering=False, debug=False,
                       num_devices=NCORES)
        with tile.TileContext(nc) as tc:
            _build_kernel(nc, tc)
        nc.compile()
        _NC_CACHE = nc
    return _NC_CACHE


def kernel(q_embed, conv_feat):
    q_embed = np.ascontiguousarray(np.asarray(q_embed), dtype=np.float32)
    conv_feat = np.ascontiguousarray(np.asarray(conv_feat), dtype=np.float32)
    assert q_embed.shape == (V, D) and conv_feat.shape == (B, D, T)

    nc = _get_nc()
    in_maps = [
        {
            "q_embed": q_embed,
            "conv_feat": np.ascontiguousarray(conv_feat[i * BPC : (i + 1) * BPC]),
        }
        for i in range(NCORES)
    ]
    try:
        res = run_bass_kernel_spmd(nc, in_maps, list(range(NCORES))).results
    except Exception:
        # transient device/RPC faults have been observed; one retry
        import time as _time

        _time.sleep(5)
        res = run_bass_kernel_spmd(nc, in_maps, list(range(NCORES))).results
    context = np.concatenate([r["context"] for r in res], axis=0)
    weights = np.concatenate([r["weights"] for r in res], axis=0)
    return context, weights


if __name__ == "__main__":
    rng = np.random.default_rng(0)
    qe = rng.standard_normal((V, D), dtype=np.float32)
    cfv = rng.standard_normal((B, D, T), dtype=np.float32)
    ctx_, w_ = kernel(qe, cfv)
    print(ctx_.shape, w_.shape, ctx_.dtype, w_.dtype)


# revision 17
# speedup vs baseline: 1.0689x; 1.0038x over previous
"""Trainium2 Bass kernel for DotProductAttention.

Reference computation (all fp32):
    sim_scores        = einsum('vd,bdt->bvt', q_embed, conv_feat)   # (B,V,T)
    attention_weights = softmax(sim_scores, axis=2)                 # over T
    context_vector    = einsum('bvt,bdt->bvd', weights, conv_feat)  # (B,V,D)
    returns (context_vector, attention_weights)

Shapes: q_embed (1000, 1024) f32, conv_feat (64, 1024, 800) f32.

Sharding: data-parallel over batch across 8 NeuronCores (8 batches/core,
q_embed replicated).

Per-core kernel design:
  - Q^T is built once on-chip via PE transposes and split into bf16 hi/lo
    parts (Q = Qhi + Qlo exactly, each bf16).  Scores are computed as
    Qhi@Chi + Qhi@Clo + Qlo@Chi (3 bf16 matmuls, fp32 PSUM accumulation)
    which carries ~1e-4 absolute error on the scores - effectively fp32
    for softmax purposes - at 3x the speed of native fp32 matmul.
  - Softmax over T (free dim): DVE reduce_max -> ACT exp(x-max) with
    fused row-sum (accum_out) -> DVE reciprocal + scale.  fp32 weights
    are DMA'd straight out; a bf16 copy feeds the second matmul.
  - Second einsum contracts over T, so both operands need T on the
    partition axis: W^T and C^T are produced by PE (tensor engine)
    transposes in bf16, then context = (W^T).T @ C^T in bf16.
"""

import sys

sys.path.insert(0, "/opt/trn_rl_repo")

import numpy as np

import concourse.bass as bass  # noqa: F401  (engine types pulled via nc)
import concourse.tile as tile
from concourse import bacc, mybir
from concourse.bass_utils import run_bass_kernel_spmd
from concourse.masks import make_identity

F32 = mybir.dt.float32
BF16 = mybir.dt.bfloat16

B, V, D, T = 64, 1000, 1024, 800
NCORES = 8
BPC = B // NCORES          # batches per core
VCH = 8                    # v chunks of 128 (last has 104 valid rows)
DCH = D // 128             # 8
TA = 400                   # stage-A t-chunk (PSUM bank limit 512 fp32)
NTA = T // TA              # 2
TCH = (T + 127) // 128     # 7 stage-B t-chunks (6x128 + 1x32)
EXP = mybir.ActivationFunctionType.Exp
AX = mybir.AxisListType.X


def _vrows(vc):
    return 128 if vc < VCH - 1 else V - 128 * (VCH - 1)  # 104 for last chunk


def _tlen(tc_):
    return 128 if tc_ < TCH - 1 else T - 128 * (TCH - 1)  # 32 for last chunk


def _build_kernel(nc, tc):
    q = nc.dram_tensor("q_embed", [V, D], F32, kind="ExternalInput")
    cf = nc.dram_tensor("conv_feat", [BPC, D, T], F32, kind="ExternalInput")
    out_ctx = nc.dram_tensor("context", [BPC, V, D], F32, kind="ExternalOutput")
    out_w = nc.dram_tensor("weights", [BPC, V, T], F32, kind="ExternalOutput")

    with (
        tc.tile_pool(name="const", bufs=1) as const,
        tc.tile_pool(name="qt", bufs=1) as qt_pool,
    ):
        ident32 = const.tile([128, 128], F32)
        make_identity(nc, ident32[:])
        identbf = const.tile([128, 128], BF16)
        make_identity(nc, identbf[:])

        # ---- Q^T hi/lo tiles [128(d), DCH, VP(v)]; filled lazily inside the
        # main loop (interleaved with batch-0 stage A so PE never waits for
        # the full q DMA before starting real work) ----
        qhiT = qt_pool.tile([128, DCH, VCH * 128], BF16)
        qloT = qt_pool.tile([128, DCH, VCH * 128], BF16)
        # zero only the padded v columns (1000..1023) -> score 0, harmless
        nc.vector.memset(qhiT[:, :, V : VCH * 128], 0.0)
        nc.vector.memset(qloT[:, :, V : VCH * 128], 0.0)

        _main_loop(nc, tc, q, cf, out_ctx, out_w, qhiT, qloT, identbf, ident32)


def _main_loop(nc, tc, q, cf, out_ctx, out_w, qhiT, qloT, identbf, ident32):
    with (
        tc.tile_pool(name="cstage", bufs=3) as cstage,
        tc.tile_pool(name="chl", bufs=2) as chl,
        tc.tile_pool(name="tposed", bufs=1) as tposed,
        tc.tile_pool(name="epool", bufs=4) as epool,
        tc.tile_pool(name="wbpool", bufs=VCH) as wbpool,
        tc.tile_pool(name="ctxpool", bufs=6) as ctxpool,
        tc.tile_pool(name="stats", bufs=3) as stats,
        tc.tile_pool(name="rcps", bufs=VCH + 1) as rcps,
        tc.tile_pool(name="psA", bufs=3, space="PSUM") as psA,
        tc.tile_pool(name="psT", bufs=3, space="PSUM") as psT,
        tc.tile_pool(name="psB", bufs=2, space="PSUM") as psB,
    ):
        # ---- main per-batch loop ----
        for b in range(BPC):
            # C (fp32) is staged in D-groups of 2x128 rows and split into
            # exact bf16 hi/lo parts (C = chi + clo up to ~2^-18 rel).
            chi = chl.tile([128, DCH, T], BF16, tag="chi")
            clo = chl.tile([128, DCH, T], BF16, tag="clo")
            cfb = cf[b].rearrange("(n p) t -> p n t", p=128)
            qsb0 = None
            if b == 0:
                # issue the first q chunk ahead of the big C staging DMAs so
                # batch-0's first transposes are not queued behind ~9us of C
                qsb0 = cstage.tile([128, D], F32, tag="qsb")
                nc.sync.dma_start(qsb0[:_vrows(0), :], q[0 : _vrows(0), :])
            for g in range(DCH // 2):
                cs = cstage.tile([128, 2, T], F32)
                nc.sync.dma_start(cs[:], cfb[:, 2 * g : 2 * g + 2, :])
                nc.gpsimd.tensor_copy(chi[:, 2 * g : 2 * g + 2, :], cs[:])
                nc.vector.tensor_sub(
                    clo[:, 2 * g : 2 * g + 2, :], cs[:], chi[:, 2 * g : 2 * g + 2, :]
                )

            wb_tiles = []
            rcp_tiles = []
            # -- stage A: scores + softmax, per v-chunk --
            for vc in range(VCH):
                rows = _vrows(vc)
                vs = slice(128 * vc, 128 * (vc + 1))
                if b == 0:
                    # Q^T setup for this v-chunk: transpose q rows into bf16
                    # hi/lo parts right before they are first needed, so the
                    # q DMA streams in parallel with batch-0 compute.
                    if vc == 0:
                        qsb = qsb0
                    else:
                        qsb = cstage.tile([128, D], F32, tag="qsb")
                        nc.sync.dma_start(
                            qsb[:rows, :], q[128 * vc : 128 * vc + rows, :]
                        )
                    for dc in range(DCH):
                        pq = psT.tile([128, 128], F32, tag="pt")
                        nc.tensor.transpose(
                            pq[:, :rows],
                            qsb[:rows, 128 * dc : 128 * (dc + 1)],
                            ident32[:rows, :rows],
                        )
                        hi = qhiT[:, dc, 128 * vc : 128 * vc + rows]
                        nc.vector.tensor_copy(hi, pq[:, :rows])
                        nc.vector.tensor_sub(
                            qloT[:, dc, 128 * vc : 128 * vc + rows],
                            pq[:, :rows], hi,
                        )
                pst = []
                for ti in range(NTA):
                    ps = psA.tile([128, TA], F32)
                    pst.append(ps)
                    tsl = slice(TA * ti, TA * (ti + 1))
                    n = 0
                    for dc in range(DCH):
                        for lhs, rhs in (
                            (qhiT, chi),
                            (qhiT, clo),
                            (qloT, chi),
                        ):
                            n += 1
                            nc.tensor.matmul(
                                ps[:],
                                lhs[:, dc, vs],
                                rhs[:, dc, tsl],
                                start=(n == 1),
                                stop=(n == 3 * DCH),
                            )
                # softmax over T for this v-chunk
                m0 = stats.tile([128, 1], F32, tag="m0")
                m1 = stats.tile([128, 1], F32, tag="m1")
                nc.vector.reduce_max(out=m0[:], in_=pst[0][:], axis=AX)
                nc.vector.reduce_max(out=m1[:], in_=pst[1][:], axis=AX)
                negm = stats.tile([128, 1], F32, tag="negm")
                nc.vector.tensor_max(negm[:], m0[:], m1[:])
                nc.vector.tensor_scalar_mul(negm[:], negm[:], -1.0)
                e = epool.tile([128, T], F32)
                s0 = stats.tile([128, 1], F32, tag="s0")
                s1 = stats.tile([128, 1], F32, tag="s1")
                nc.scalar.activation(
                    out=e[:, 0:TA], in_=pst[0][:], func=EXP, bias=negm[:],
                    accum_out=s0[:],
                )
                nc.scalar.activation(
                    out=e[:, TA:T], in_=pst[1][:], func=EXP, bias=negm[:],
                    accum_out=s1[:],
                )
                # stage B uses UNNORMALIZED exp in bf16 (available right after
                # exp); 1/sum is folded into the context PSUM->SBUF copy as a
                # per-partition scale, so the sum/reciprocal/normalize chain
                # stays off the PE critical path.
                wb = wbpool.tile([128, T], BF16)
                nc.gpsimd.tensor_copy(wb[:], e[:])
                wb_tiles.append(wb)
                rcp = rcps.tile([128, 1], F32, tag="rcp")
                nc.vector.tensor_add(rcp[:], s0[:], s1[:])
                nc.vector.reciprocal(rcp[:], rcp[:])
                rcp_tiles.append(rcp)
                nc.vector.tensor_scalar_mul(e[:], e[:], rcp[:])
                nc.sync.dma_start(out_w[b, 128 * vc : 128 * vc + rows, :], e[:rows, :])

            # -- transposes (PE) to put T on partitions --
            # 8 [*,128] blocks land side by side in one PSUM bank, then one
            # wide DVE copy moves the whole bank to SBUF.
            ct = tposed.tile([128, TCH, D], BF16, tag="ct")
            for tch in range(TCH):
                tl = _tlen(tch)
                pt = psT.tile([128, DCH, 128], BF16, tag="pt")
                for dc in range(DCH):
                    nc.tensor.transpose(
                        pt[:tl, dc, :],
                        chi[:, dc, 128 * tch : 128 * tch + tl],
                        identbf[:],
                    )
                nc.vector.tensor_copy(
                    ct[:tl, tch, :], pt[:tl].rearrange("p a b -> p (a b)")
                )
            wt = tposed.tile([128, TCH, VCH * 128], BF16, tag="wt")
            for tch in range(TCH):
                tl = _tlen(tch)
                pt = psT.tile([128, VCH, 128], BF16, tag="pt")
                for vc in range(VCH):
                    nc.tensor.transpose(
                        pt[:tl, vc, :],
                        wb_tiles[vc][:, 128 * tch : 128 * tch + tl],
                        identbf[:],
                    )
                nc.vector.tensor_copy(
                    wt[:tl, tch, :], pt[:tl].rearrange("p a b -> p (a b)")
                )

            # -- stage B: context = W @ C^T, contraction over T --
            for vc in range(VCH):
                rows = _vrows(vc)
                vs = slice(128 * vc, 128 * (vc + 1))
                for dc2 in range(2):
                    ps = psB.tile([128, 512], F32)
                    for tch in range(TCH):
                        tl = _tlen(tch)
                        nc.tensor.matmul(
                            ps[:],
                            wt[:tl, tch, vs],
                            ct[:tl, tch, 512 * dc2 : 512 * (dc2 + 1)],
                            start=(tch == 0),
                            stop=(tch == TCH - 1),
                        )
                    octx = ctxpool.tile([128, 512], F32)
                    nc.scalar.mul(octx[:], ps[:], rcp_tiles[vc][:])
                    nc.sync.dma_start(
                        out_ctx[b, 128 * vc : 128 * vc + rows,
                                512 * dc2 : 512 * (dc2 + 1)],
                        octx[:rows, :],
                    )


_NC_CACHE = None


def _get_nc():
    global _NC_CACHE
    if _NC_CACHE is None:
        nc = bacc.Bacc("TRN2", target_bir_lowering=False, debug=False,
                       num_devices=NCORES)
        with tile.TileContext(nc) as tc:
            _build_kernel(nc, tc)
        nc.compile()
        _NC_CACHE = nc
    return _NC_CACHE


def kernel(q_embed, conv_feat):
    q_embed = np.ascontiguousarray(np.asarray(q_embed), dtype=np.float32)
    conv_feat = np.ascontiguousarray(np.asarray(conv_feat), dtype=np.float32)
    assert q_embed.shape == (V, D) and conv_feat.shape == (B, D, T)

    nc = _get_nc()
    in_maps = [
        {
            "q_embed": q_embed,
            "conv_feat": np.ascontiguousarray(conv_feat[i * BPC : (i + 1) * BPC]),
        }
        for i in range(NCORES)
    ]
    try:
        res = run_bass_kernel_spmd(nc, in_maps, list(range(NCORES))).results
    except Exception:
        # transient device/RPC faults have been observed; one retry
        import time as _time

        _time.sleep(5)
        res = run_bass_kernel_spmd(nc, in_maps, list(range(NCORES))).results
    context = np.concatenate([r["context"] for r in res], axis=0)
    weights = np.concatenate([r["weights"] for r in res], axis=0)
    return context, weights


if __name__ == "__main__":
    rng = np.random.default_rng(0)
    qe = rng.standard_normal((V, D), dtype=np.float32)
    cfv = rng.standard_normal((B, D, T), dtype=np.float32)
    ctx_, w_ = kernel(qe, cfv)
    print(ctx_.shape, w_.shape, ctx_.dtype, w_.dtype)
